# revision 35
# baseline (speedup 1.0000x reference)
"""Trainium2 Bass kernel for nn_HadaMard: fused proj + 2xLayerNorm + outer product.

Reference computation (per batch b):
  qf = q[b].reshape(C1, N)           # [1024, 1024]  (C1 rows, N=H*W cols)
  proj = Wp @ qf + bp                # [256, 1024]
  qn = LN_over_d(proj) * g1 + b1     # LN over the 256-channel dim
  xn = LN_over_e(x[b]) * g2 + b2     # LN over the 32-channel dim
  out[d*32+e, n] = qn[d, n] * xn[e, n]   # [8192, 1024]

Sharding: data-parallel over B=8, one batch per NeuronCore.

Layout ("flipped tiling"): output tiles keep qn's channel dim d on the
partitions (dblock in {0,1} x 128 partitions) and iterate e in the free dim.
  - proj: PE matmuls (bf16), accumulated in f32 PSUM, k-loop ordered by
    DMA arrival; the q-stats/LN chain is pipelined by 512-column halves
    so qn's first half is ready early.
  - LN stats via bf16 ones-matmuls; 1/sd via reciprocal_approx_fast.
  - xn (32 rows, bf16) replicated to 128 partitions via DRAM-roundtrip
    DMAs with stride-0 source (partition_broadcast); one scratch copy per
    issuing engine keeps the read ordered behind the write in-queue.
  - product: all-bf16 tensor_tensor multiplies (DVE 2x mode) with the qn
    operand repeated along the free dim via a stride-0 AP; ~1/3 of the
    chunks run on the Pool engine.
  - output: bf16 DRAM tensor (host converts to f32), 4-e-wide tiles,
    DMAs spread across SP / Act / Pool.

Axon-backend constraints honored: no float32r matmuls, no AluOp.divide,
at most one PSUM operand per DVE op, no PSUM operands on Pool, DMA only
on SP / Act / Pool.
"""

import numpy as np

_CACHE = {}

B, C1, H, W = 8, 1024, 32, 32
C2 = 32
Cp = 256
N = 1024
CD = Cp * C2  # 8192
MD = Cp // 128  # 2 row-blocks of proj/qn
EPS = 1e-5

# mul chunks (e0, e1) per dblock for DVE ('v') and Pool ('g').
# DVE chunks are emitted per column-half; Pool chunks are full-width.
_MUL_V = {
    0: [(0, 2), (2, 4), (4, 8), (8, 12), (12, 16), (16, 20), (20, 24),
        (24, 28), (28, 32)],
    1: [(12, 16), (16, 20), (20, 24)],
}
_MUL_G = {
    0: [],
    1: [(0, 2), (2, 4), (4, 6), (6, 8), (8, 10), (10, 12), (24, 26), (26, 28),
        (28, 30), (30, 32)],
}
# xn broadcast chunks: (e0, e1, engine): 's' SP, 'a' Act, 'g' Pool
_BCAST = [(0, 2, "s"), (2, 4, "s"), (4, 8, "s"), (8, 12, "s"), (12, 16, "s"),
          (16, 20, "a"), (20, 24, "a"), (24, 32, "g")]
# output tiles per dblock: 8 x 4-e tiles, (j -> dma engine)
_OUT_ENG = {
    0: ["s", "a", "s", "a", "s", "a", "s", "a"],
    1: ["a", "g", "s", "a", "g", "a", "g", "s"],
}
# O-tile allocation order (rough completion order; pool bufs=6)
_ALLOC_ORDER = [(1, 0), (1, 1), (0, 0), (1, 2), (0, 1), (1, 3), (0, 2),
                (0, 3), (1, 4), (0, 4), (1, 5), (0, 5), (1, 6), (0, 6),
                (1, 7), (0, 7)]
# q-tile k -> load engine; proj accumulation follows arrival order
_Q_ENG = {0: "s", 3: "s", 1: "a", 4: "a", 2: "g", 5: "g", 6: "g", 7: "g"}
_K_ORDER = [2, 0, 1, 5, 3, 4, 6, 7]


def _build_nc(simple):
    import concourse.bacc as bacc
    import concourse.bass as bass
    import concourse.mybir as mybir
    import concourse.tile as tile

    F32 = mybir.dt.float32
    BF16 = mybir.dt.bfloat16
    MULT = mybir.AluOpType.mult
    SUB = mybir.AluOpType.subtract
    ADD = mybir.AluOpType.add
    SQRT = mybir.ActivationFunctionType.Sqrt

    nc = bacc.Bacc(None, target_bir_lowering=False)

    q_d = nc.dram_tensor("q", [C1, N], BF16, kind="ExternalInput")
    w_d = nc.dram_tensor("w", [C1, Cp], BF16, kind="ExternalInput")
    x_d = nc.dram_tensor("x", [C2, N], BF16, kind="ExternalInput")
    if not simple:
        bp_d = nc.dram_tensor("bpc", [128, MD], F32, kind="ExternalInput")
        g1_d = nc.dram_tensor("g1c", [128, MD], F32, kind="ExternalInput")
        b1_d = nc.dram_tensor("b1c", [128, MD], F32, kind="ExternalInput")
        g2_d = nc.dram_tensor("g2r", [C2, 1], F32, kind="ExternalInput")
        b2_d = nc.dram_tensor("b2r", [C2, 1], F32, kind="ExternalInput")
    xs_d = {
        "s": nc.dram_tensor("xs0", [C2, N], BF16, kind="ExternalOutput"),
        "a": nc.dram_tensor("xs1", [C2, N], BF16, kind="ExternalOutput"),
        "g": nc.dram_tensor("xs2", [C2, N], BF16, kind="ExternalOutput"),
    }
    out_d = nc.dram_tensor("out", [CD, N], BF16, kind="ExternalOutput")

    def rep_ap(t, r, h=None):
        """qn tile AP repeated r times along a stride-0 free dim.

        h=None: full rows; h=0/1: 512-column half (offset 512h)."""
        a = t[:].copy()
        while len(a.ap) > 0:
            a.ap.pop()
        a.ap.append([N, 128])
        a.ap.append([0, r])
        if h is None:
            a.ap.append([1, N])
        else:
            a.ap.append([1, 512])
            a.offset = a.offset + 512 * h
        return a

    def sub_ap(t, e0, e1, h=None):
        """e-major tile viewed as (p, e, n): slice e and optionally a
        512-col half of n.  Partition stride taken from the tile itself."""
        a = t[:].copy()
        base = a.offset
        pstride = a.ap[0][0]
        while len(a.ap) > 0:
            a.ap.pop()
        a.ap.append([pstride, 128])
        a.ap.append([N, e1 - e0])
        if h is None:
            a.ap.append([1, N])
            a.offset = base + e0 * N
        else:
            a.ap.append([1, 512])
            a.offset = base + e0 * N + 512 * h
        return a

    with tile.TileContext(nc) as tc:
        with (
            tc.tile_pool(name="cst", bufs=1) as cst,
            tc.tile_pool(name="qp", bufs=1) as qp,
            tc.tile_pool(name="wrk", bufs=1) as wrk,
            tc.tile_pool(name="bp16", bufs=1) as bp16,
            tc.tile_pool(name="keep", bufs=1) as keep,
            tc.tile_pool(name="xbe", bufs=1) as xbep,
            tc.tile_pool(name="op", bufs=9) as op,
            tc.tile_pool(name="ps", bufs=4, space=bass.MemorySpace.PSUM) as ps,
        ):
            eng = {"s": nc.sync, "a": nc.scalar, "g": nc.gpsimd}

            # ---------- constants / memsets (DVE) ----------
            onesx = cst.tile([C2, C2], BF16, tag="onesx")
            nc.vector.memset(onesx[:], 1.0 / C2)
            onesq = cst.tile([128, 128], BF16, tag="onesq")
            nc.vector.memset(onesq[:], 1.0 / Cp)
            eps_t = cst.tile([128, 1], F32, tag="eps")
            nc.vector.memset(eps_t[:], EPS)

            _wn = [0]

            def wtile():
                t = wrk.tile([128, N], F32, tag=f"t{_wn[0] % 5}")
                _wn[0] += 1
                return t

            halves = [slice(0, 512), slice(512, 1024)]

            # ---------- input loads ----------
            # SP: w0, q0, x, q3; Act: w1, q1, q4; Pool: q2, q5, q6, q7
            wg = []
            for g in range(2):
                t = cst.tile([128, 4 * Cp], BF16, tag=f"w{g}")
                dst = t[:].rearrange("p (k d) -> p k d", k=4)
                src = w_d[512 * g : 512 * (g + 1), :].rearrange(
                    "(k p) d -> p k d", k=4
                )
                [nc.sync, nc.scalar][g].dma_start(dst, src)
                wg.append(t)
            xsb = cst.tile([C2, N], BF16, tag="x")
            nc.sync.dma_start(xsb[:], x_d[:])
            q_sb = {}
            for k in [2, 0, 1, 5, 3, 4, 6, 7]:
                t = qp.tile([128, N], BF16, tag=f"q{k}")
                eng[_Q_ENG[k]].dma_start(t[:], q_d[128 * k : 128 * (k + 1), :])
                q_sb[k] = t
            if not simple:
                bp_sb = cst.tile([128, MD], F32, tag="bp")
                nc.sync.dma_start(bp_sb[:], bp_d[:])
                g1_sb = cst.tile([128, MD], F32, tag="g1")
                nc.sync.dma_start(g1_sb[:], g1_d[:])
                b1_sb = cst.tile([128, MD], F32, tag="b1")
                nc.scalar.dma_start(b1_sb[:], b1_d[:])
                g2_sb = cst.tile([C2, 1], F32, tag="g2")
                nc.scalar.dma_start(g2_sb[:], g2_d[:])
                b2_sb = cst.tile([C2, 1], F32, tag="b2")
                nc.scalar.dma_start(b2_sb[:], b2_d[:])

            # ---------- x stats + first part of proj (PE) ----------
            xsq = bp16.tile([C2, N], BF16, tag="xsq")
            nc.vector.tensor_tensor(xsq[:], xsb[:], xsb[:], op=MULT)
            mx_ps = ps.tile([128, N], F32, tag="ps")
            mxq_ps = ps.tile([128, N], F32, tag="ps")

            proj = []
            for md in range(MD):
                pj = ps.tile([128, N], F32, tag="ps")
                proj.append(pj)

            def proj_mms(i, k):
                for hs in halves:
                    for md in range(MD):
                        lh = wg[k // 4][:, (k % 4) * Cp + 128 * md :
                                        (k % 4) * Cp + 128 * (md + 1)]
                        nc.tensor.matmul(proj[md][:, hs], lh, q_sb[k][:, hs],
                                         start=(i == 0), stop=(i == 7))

            # PE queue: proj[k2, k0], x-stat mms, proj[rest]
            proj_mms(0, _K_ORDER[0])
            proj_mms(1, _K_ORDER[1])
            for hs in halves:
                nc.tensor.matmul(mx_ps[:C2, hs], onesx[:], xsb[:, hs],
                                 start=True, stop=True)
            for hs in halves:
                nc.tensor.matmul(mxq_ps[:C2, hs], onesx[:], xsq[:, hs],
                                 start=True, stop=True)
            for i in range(2, 8):
                proj_mms(i, _K_ORDER[i])

            # ---------- x LN (half-pipelined chain) ----------
            mx2 = wtile()
            xd = wtile()
            varx = wtile()
            sdx = wtile()
            rsdx = keep.tile([C2, N], F32, tag="rsdx")
            xn = keep.tile([C2, N], BF16, tag="xn")
            if not simple:
                xtmp = wtile()
            for hs in halves:
                nc.scalar.square(mx2[:C2, hs], mx_ps[:C2, hs])
                nc.vector.tensor_tensor(xd[:C2, hs], xsb[:, hs],
                                        mx_ps[:C2, hs], op=SUB)
                nc.vector.tensor_tensor(varx[:C2, hs], mxq_ps[:C2, hs],
                                        mx2[:C2, hs], op=SUB)
                nc.scalar.activation(sdx[:C2, hs], varx[:C2, hs], SQRT,
                                     bias=eps_t[:C2, :])
                nc.vector.reciprocal_approx_fast(rsdx[:, hs], sdx[:C2, hs])
                if simple:
                    nc.vector.tensor_tensor(xn[:, hs], xd[:C2, hs],
                                            rsdx[:, hs], op=MULT)
                else:
                    nc.vector.tensor_tensor(xtmp[:C2, hs], xd[:C2, hs],
                                            rsdx[:, hs], op=MULT)
                    nc.vector.tensor_scalar(xn[:, hs], xtmp[:C2, hs],
                                            g2_sb[:], b2_sb[:],
                                            op0=MULT, op1=ADD)

            # ---------- xn scratch writes + stride-0 broadcasts ----------
            xbe = {}
            _bc_written = set()

            def emit_bcast(which):
                for e0, e1, en in _BCAST:
                    if en != which:
                        continue
                    if which not in _bc_written:
                        eng[which].dma_start(xs_d[which][:], xn[:])
                        _bc_written.add(which)
                    t = xbep.tile([128, (e1 - e0) * N], BF16,
                                  tag=f"xbe{e0}")
                    eng[which].dma_start(
                        t[:], xs_d[which][e0:e1, :].partition_broadcast(128))
                    xbe[(e0, e1)] = t

            emit_bcast("s")
            emit_bcast("g")

            # ---------- q stats, pipelined by 512-column halves ----------
            pb, sq, diff, qnb, dvk = [], [], [], [], []
            for md in range(MD):
                pbt = bp16.tile([128, N], BF16, tag=f"pb{md}")
                pb.append(pbt)
                sqt = bp16.tile([128, N], BF16, tag=f"sq{md}")
                sq.append(sqt)
            mean_ps = ps.tile([128, N], F32, tag="ps")
            msq_ps = ps.tile([128, N], F32, tag="ps")
            mb2 = wtile()
            var = wtile()
            sd = wtile()
            for md in range(MD):
                dft = wtile()
                diff.append(dft)
            rsd = keep.tile([128, N], F32, tag="rsd")
            for md in range(MD):
                qnt = keep.tile([128, N], BF16, tag=f"qn{md}")
                qnb.append(qnt)
            qnb_pool = keep.tile([128, N], BF16, tag="qnp")
            if not simple:
                for md in range(MD):
                    dvt = keep.tile([128, N], F32, tag=f"dv{md}")
                    dvk.append(dvt)

            # ---------- product helpers ----------
            def xbe_of(e0, e1):
                for (b0, b1), t in xbe.items():
                    if b0 <= e0 and e1 <= b1:
                        return t, b0
                raise AssertionError((e0, e1))

            out_view = []
            for md in range(MD):
                ov = out_d[4096 * md : 4096 * (md + 1), :].rearrange(
                    "(p e) n -> p e n", e=32
                )
                out_view.append(ov)

            otile = {}
            for md, j in _ALLOC_ORDER:
                ot = op.tile([128, 4 * N], BF16, tag="ot")
                otile[(md, j)] = ot

            def emit_mul(e_, qsrc, md, e0, e1, h=None):
                j = e0 // 4
                assert e1 <= 4 * (j + 1)
                o = otile[(md, j)]
                xt, b0 = xbe_of(e0, e1)
                e_.tensor_tensor(
                    sub_ap(o, e0 - 4 * j, e1 - 4 * j, h)
                    if h is not None
                    else o[:, (e0 - 4 * j) * N : (e1 - 4 * j) * N],
                    rep_ap(qsrc, e1 - e0, h),
                    sub_ap(xt, e0 - b0, e1 - b0, h),
                    op=MULT)

            def emit_out(md, j):
                o = otile[(md, j)]
                eng[_OUT_ENG[md][j]].dma_start(
                    out_view[md][:, 4 * j : 4 * (j + 1), :], o[:])

            # DVE mul order within a half: by e (broadcast arrival order)
            vseq = sorted(
                [(0, c) for c in _MUL_V[0]] + [(1, c) for c in _MUL_V[1]],
                key=lambda mc: (mc[1][0], mc[0]))

            # ---------- stats chain + muls, pipelined by halves ----------
            def emit_pb_sq(hs):
                for md in range(MD):
                    if simple:
                        nc.scalar.copy(pb[md][:, hs], proj[md][:, hs])
                    else:
                        nc.vector.tensor_scalar(pb[md][:, hs],
                                                proj[md][:, hs],
                                                bp_sb[:, md : md + 1], None,
                                                op0=ADD)
                    e_ = nc.vector if md == 0 else nc.gpsimd
                    e_.tensor_tensor(sq[md][:, hs], pb[md][:, hs],
                                     pb[md][:, hs], op=MULT)

            def emit_mms(hs):
                for md in range(MD):
                    nc.tensor.matmul(mean_ps[:, hs], onesq[:], pb[md][:, hs],
                                     start=(md == 0), stop=(md == MD - 1))
                for md in range(MD):
                    nc.tensor.matmul(msq_ps[:, hs], onesq[:], sq[md][:, hs],
                                     start=(md == 0), stop=(md == MD - 1))

            def emit_chain(hs):
                nc.scalar.square(mb2[:, hs], mean_ps[:, hs])
                nc.vector.tensor_tensor(diff[0][:, hs], pb[0][:, hs],
                                        mean_ps[:, hs], op=SUB)
                nc.vector.tensor_tensor(var[:, hs], msq_ps[:, hs],
                                        mb2[:, hs], op=SUB)
                nc.scalar.activation(sd[:, hs], var[:, hs], SQRT,
                                     bias=eps_t[:])
                nc.vector.tensor_tensor(diff[1][:, hs], pb[1][:, hs],
                                        mean_ps[:, hs], op=SUB)
                nc.vector.reciprocal_approx_fast(rsd[:, hs], sd[:, hs])
                if simple:
                    nc.vector.tensor_tensor(qnb[0][:, hs], diff[0][:, hs],
                                            rsd[:, hs], op=MULT)
                    # qn[1] lives only in Pool's copy; DVE md1 muls read it
                    nc.gpsimd.tensor_tensor(qnb_pool[:, hs], diff[1][:, hs],
                                            rsd[:, hs], op=MULT)
                else:
                    for md in range(MD):
                        nc.vector.tensor_tensor(dvk[md][:, hs],
                                                diff[md][:, hs],
                                                rsd[:, hs], op=MULT)
                        nc.vector.tensor_scalar(qnb[md][:, hs],
                                                dvk[md][:, hs],
                                                g1_sb[:, md : md + 1],
                                                b1_sb[:, md : md + 1],
                                                op0=MULT, op1=ADD)
                    nc.gpsimd.tensor_copy(qnb_pool[:, hs], qnb[1][:, hs])

            def emit_muls(hi):
                q1src = qnb_pool if simple else qnb[1]
                with tc.high_priority():
                    for e0, e1 in _MUL_G[1]:
                        emit_mul(nc.gpsimd, qnb_pool, 1, e0, e1, hi)
                    for md, (e0, e1) in vseq:
                        emit_mul(nc.vector,
                                 qnb[0] if md == 0 else q1src, md, e0, e1, hi)

            h0, h1 = halves
            emit_pb_sq(h0)
            emit_mms(h0)
            emit_chain(h0)
            # the whole h1 stats chain is emitted before the Act broadcasts
            # so the bcast DMAs can't block sd-h1 / qnb-h1 on the Act queue,
            # and Pool's full-width muls (needing both qnb_pool halves)
            # unblock as early as possible
            emit_pb_sq(h1)
            emit_mms(h1)
            emit_chain(h1)
            emit_bcast("a")
            emit_muls(0)
            emit_muls(1)

            # output DMAs
            for md, j in _ALLOC_ORDER:
                emit_out(md, j)

    nc.compile()
    return nc


def _host_inputs(q, x, Wp, bp, g1, b1, g2, b2):
    """Build the 8 per-core input maps."""
    import os

    import ml_dtypes

    simple = os.environ.get("HM_SIMPLE", "0") == "1"
    qf = np.asarray(q, dtype=np.float32).reshape(B, C1, N)
    qb = np.ascontiguousarray(qf).astype(ml_dtypes.bfloat16)
    xf = np.ascontiguousarray(
        np.asarray(x, dtype=np.float32).reshape(B, C2, N)
    ).astype(ml_dtypes.bfloat16)
    wpt = np.ascontiguousarray(np.asarray(Wp, dtype=np.float32).T).astype(
        ml_dtypes.bfloat16
    )
    in_maps = []
    for b in range(B):
        m = {
            "q": np.ascontiguousarray(qb[b]),
            "w": wpt,
            "x": np.ascontiguousarray(xf[b]),
        }
        if not simple:
            m["bpc"] = np.ascontiguousarray(
                np.asarray(bp, dtype=np.float32).reshape(MD, 128).T)
            m["g1c"] = np.ascontiguousarray(
                np.asarray(g1, dtype=np.float32).reshape(MD, 128).T)
            m["b1c"] = np.ascontiguousarray(
                np.asarray(b1, dtype=np.float32).reshape(MD, 128).T)
            m["g2r"] = np.ascontiguousarray(
                np.asarray(g2, dtype=np.float32)[:, None])
            m["b2r"] = np.ascontiguousarray(
                np.asarray(b2, dtype=np.float32)[:, None])
        in_maps.append(m)
    return in_maps


def _run(in_maps, trace=False):
    import os

    from concourse.bass_utils import run_bass_kernel_spmd

    key = "nc" + os.environ.get("HM_SIMPLE", "0")
    if key not in _CACHE:
        _CACHE[key] = _build_nc(os.environ.get("HM_SIMPLE", "0") == "1")
    nc = _CACHE[key]
    res = run_bass_kernel_spmd(nc, in_maps, core_ids=list(range(B)), trace=trace)
    return res


def kernel(q, x, Wp, bp, g1, b1, g2, b2):
    import os

    simple = (
        np.allclose(np.asarray(bp), 0)
        and np.allclose(np.asarray(g1), 1)
        and np.allclose(np.asarray(b1), 0)
        and np.allclose(np.asarray(g2), 1)
        and np.allclose(np.asarray(b2), 0)
    )
    os.environ["HM_SIMPLE"] = "1" if simple else "0"
    in_maps = _host_inputs(q, x, Wp, bp, g1, b1, g2, b2)
    res = _run(in_maps, trace=False)
    out = np.stack(
        [
            np.asarray(res.results[b]["out"]).astype(np.float32).reshape(CD, H, W)
            for b in range(B)
        ]
    )
    _CACHE["last_res"] = res
    return out


# revision 41
# speedup vs baseline: 1.0081x; 1.0081x over previous
"""Trainium2 Bass kernel for nn_HadaMard: fused proj + 2xLayerNorm + outer product.

Reference computation (per batch b):
  qf = q[b].reshape(C1, N)           # [1024, 1024]  (C1 rows, N=H*W cols)
  proj = Wp @ qf + bp                # [256, 1024]
  qn = LN_over_d(proj) * g1 + b1     # LN over the 256-channel dim
  xn = LN_over_e(x[b]) * g2 + b2     # LN over the 32-channel dim
  out[d*32+e, n] = qn[d, n] * xn[e, n]   # [8192, 1024]

Sharding: data-parallel over B=8, one batch per NeuronCore.

Layout ("flipped tiling"): output tiles keep qn's channel dim d on the
partitions (dblock in {0,1} x 128 partitions) and iterate e in the free dim.
  - proj: PE matmuls (bf16), accumulated in f32 PSUM, k-loop ordered by
    DMA arrival; the q-stats/LN chain is pipelined by 512-column halves
    so qn's first half is ready early.
  - LN stats via bf16 ones-matmuls; 1/sd via reciprocal_approx_fast.
  - xn (32 rows, bf16) replicated to 128 partitions via DRAM-roundtrip
    DMAs with stride-0 source (partition_broadcast); one scratch copy per
    issuing engine keeps the read ordered behind the write in-queue.
  - product: all-bf16 tensor_tensor multiplies (DVE 2x mode) with the qn
    operand repeated along the free dim via a stride-0 AP; ~1/3 of the
    chunks run on the Pool engine.
  - output: bf16 DRAM tensor (host converts to f32), 4-e-wide tiles,
    DMAs spread across SP / Act / Pool.

Axon-backend constraints honored: no float32r matmuls, no AluOp.divide,
at most one PSUM operand per DVE op, no PSUM operands on Pool, DMA only
on SP / Act / Pool.
"""

import numpy as np

_CACHE = {}

B, C1, H, W = 8, 1024, 32, 32
C2 = 32
Cp = 256
N = 1024
CD = Cp * C2  # 8192
MD = Cp // 128  # 2 row-blocks of proj/qn
EPS = 1e-5

# mul chunks (e0, e1) per dblock for DVE ('v') and Pool ('g').
# DVE chunks are emitted per column-half; Pool chunks are full-width.
_MUL_V = {
    0: [(0, 2), (2, 4), (4, 8), (8, 12), (12, 16), (16, 20), (20, 24),
        (24, 28), (28, 30), (30, 32)],
    1: [(12, 16), (16, 20), (20, 24)],
}
_MUL_G = {
    0: [],
    1: [(0, 2), (2, 4), (4, 6), (6, 8), (8, 10), (10, 12), (24, 26), (26, 28),
        (28, 30), (30, 32)],
}
# xn broadcast chunks: (e0, e1, engine): 's' SP, 'a' Act, 'g' Pool
_BCAST = [(0, 2, "s"), (2, 4, "s"), (4, 8, "s"), (8, 12, "s"), (12, 16, "s"),
          (16, 20, "a"), (20, 24, "a"), (24, 28, "g"), (28, 32, "g")]
# output tiles per dblock: 8 x 4-e tiles, (j -> dma engine)
_OUT_ENG = {
    0: ["s", "a", "s", "a", "s", "a", "s", "a"],
    1: ["a", "g", "s", "a", "g", "a", "g", "s"],
}
# O-tile allocation order (rough completion order; pool bufs=6)
_ALLOC_ORDER = [(1, 0), (1, 1), (0, 0), (1, 2), (0, 1), (1, 3), (0, 2),
                (0, 3), (1, 4), (0, 4), (1, 5), (0, 5), (1, 6), (0, 6),
                (1, 7), (0, 7)]
# q-tile k -> load engine; proj accumulation follows arrival order
_Q_ENG = {0: "s", 3: "s", 1: "a", 4: "a", 2: "g", 5: "g", 6: "g", 7: "g"}
_K_ORDER = [2, 0, 1, 5, 3, 4, 6, 7]


def _build_nc(simple):
    import concourse.bacc as bacc
    import concourse.bass as bass
    import concourse.mybir as mybir
    import concourse.tile as tile

    F32 = mybir.dt.float32
    BF16 = mybir.dt.bfloat16
    MULT = mybir.AluOpType.mult
    SUB = mybir.AluOpType.subtract
    ADD = mybir.AluOpType.add
    SQRT = mybir.ActivationFunctionType.Sqrt

    nc = bacc.Bacc(None, target_bir_lowering=False)

    q_d = nc.dram_tensor("q", [C1, N], BF16, kind="ExternalInput")
    w_d = nc.dram_tensor("w", [C1, Cp], BF16, kind="ExternalInput")
    x_d = nc.dram_tensor("x", [C2, N], BF16, kind="ExternalInput")
    if not simple:
        bp_d = nc.dram_tensor("bpc", [128, MD], F32, kind="ExternalInput")
        g1_d = nc.dram_tensor("g1c", [128, MD], F32, kind="ExternalInput")
        b1_d = nc.dram_tensor("b1c", [128, MD], F32, kind="ExternalInput")
        g2_d = nc.dram_tensor("g2r", [C2, 1], F32, kind="ExternalInput")
        b2_d = nc.dram_tensor("b2r", [C2, 1], F32, kind="ExternalInput")
    xs_d = {
        "s": nc.dram_tensor("xs0", [C2, N], BF16, kind="ExternalOutput"),
        "a": nc.dram_tensor("xs1", [C2, N], BF16, kind="ExternalOutput"),
        "g": nc.dram_tensor("xs2", [C2, N], BF16, kind="ExternalOutput"),
    }
    out_d = nc.dram_tensor("out", [CD, N], BF16, kind="ExternalOutput")

    def rep_ap(t, r, h=None):
        """qn tile AP repeated r times along a stride-0 free dim.

        h=None: full rows; h=0/1: 512-column half (offset 512h)."""
        a = t[:].copy()
        while len(a.ap) > 0:
            a.ap.pop()
        a.ap.append([N, 128])
        a.ap.append([0, r])
        if h is None:
            a.ap.append([1, N])
        else:
            a.ap.append([1, 512])
            a.offset = a.offset + 512 * h
        return a

    def sub_ap(t, e0, e1, h=None):
        """e-major tile viewed as (p, e, n): slice e and optionally a
        512-col half of n.  Partition stride taken from the tile itself."""
        a = t[:].copy()
        base = a.offset
        pstride = a.ap[0][0]
        while len(a.ap) > 0:
            a.ap.pop()
        a.ap.append([pstride, 128])
        a.ap.append([N, e1 - e0])
        if h is None:
            a.ap.append([1, N])
            a.offset = base + e0 * N
        else:
            a.ap.append([1, 512])
            a.offset = base + e0 * N + 512 * h
        return a

    with tile.TileContext(nc) as tc:
        with (
            tc.tile_pool(name="cst", bufs=1) as cst,
            tc.tile_pool(name="qp", bufs=1) as qp,
            tc.tile_pool(name="wrk", bufs=1) as wrk,
            tc.tile_pool(name="bp16", bufs=1) as bp16,
            tc.tile_pool(name="keep", bufs=1) as keep,
            tc.tile_pool(name="xbe", bufs=1) as xbep,
            tc.tile_pool(name="op", bufs=9) as op,
            tc.tile_pool(name="ps", bufs=4, space=bass.MemorySpace.PSUM) as ps,
        ):
            eng = {"s": nc.sync, "a": nc.scalar, "g": nc.gpsimd}

            # ---------- constants / memsets (DVE) ----------
            onesx = cst.tile([C2, C2], BF16, tag="onesx")
            nc.vector.memset(onesx[:], 1.0 / C2)
            onesq = cst.tile([128, 128], BF16, tag="onesq")
            nc.vector.memset(onesq[:], 1.0 / Cp)
            eps_t = cst.tile([128, 1], F32, tag="eps")
            nc.vector.memset(eps_t[:], EPS)

            _wn = [0]

            def wtile():
                t = wrk.tile([128, N], F32, tag=f"t{_wn[0] % 5}")
                _wn[0] += 1
                return t

            halves = [slice(0, 512), slice(512, 1024)]

            # ---------- input loads ----------
            # SP: w0, q0, x, q3; Act: w1, q1, q4; Pool: q2, q5, q6, q7
            wg = []
            for g in range(2):
                t = cst.tile([128, 4 * Cp], BF16, tag=f"w{g}")
                dst = t[:].rearrange("p (k d) -> p k d", k=4)
                src = w_d[512 * g : 512 * (g + 1), :].rearrange(
                    "(k p) d -> p k d", k=4
                )
                [nc.sync, nc.scalar][g].dma_start(dst, src)
                wg.append(t)
            xsb = cst.tile([C2, N], BF16, tag="x")
            nc.sync.dma_start(xsb[:], x_d[:])
            q_sb = {}
            for k in [2, 0, 1, 5, 3, 4, 6, 7]:
                t = qp.tile([128, N], BF16, tag=f"q{k}")
                eng[_Q_ENG[k]].dma_start(t[:], q_d[128 * k : 128 * (k + 1), :])
                q_sb[k] = t
            if not simple:
                bp_sb = cst.tile([128, MD], F32, tag="bp")
                nc.sync.dma_start(bp_sb[:], bp_d[:])
                g1_sb = cst.tile([128, MD], F32, tag="g1")
                nc.sync.dma_start(g1_sb[:], g1_d[:])
                b1_sb = cst.tile([128, MD], F32, tag="b1")
                nc.scalar.dma_start(b1_sb[:], b1_d[:])
                g2_sb = cst.tile([C2, 1], F32, tag="g2")
                nc.scalar.dma_start(g2_sb[:], g2_d[:])
                b2_sb = cst.tile([C2, 1], F32, tag="b2")
                nc.scalar.dma_start(b2_sb[:], b2_d[:])

            # ---------- x stats + first part of proj (PE) ----------
            xsq = bp16.tile([C2, N], BF16, tag="xsq")
            nc.vector.tensor_tensor(xsq[:], xsb[:], xsb[:], op=MULT)
            mx_ps = ps.tile([128, N], F32, tag="ps")
            mxq_ps = ps.tile([128, N], F32, tag="ps")

            proj = []
            for md in range(MD):
                pj = ps.tile([128, N], F32, tag="ps")
                proj.append(pj)

            def proj_mms(i, k):
                for hs in halves:
                    for md in range(MD):
                        lh = wg[k // 4][:, (k % 4) * Cp + 128 * md :
                                        (k % 4) * Cp + 128 * (md + 1)]
                        nc.tensor.matmul(proj[md][:, hs], lh, q_sb[k][:, hs],
                                         start=(i == 0), stop=(i == 7))

            # PE queue: proj[k2, k0], x-stat mms, proj[rest]
            proj_mms(0, _K_ORDER[0])
            proj_mms(1, _K_ORDER[1])
            for hs in halves:
                nc.tensor.matmul(mx_ps[:C2, hs], onesx[:], xsb[:, hs],
                                 start=True, stop=True)
            for hs in halves:
                nc.tensor.matmul(mxq_ps[:C2, hs], onesx[:], xsq[:, hs],
                                 start=True, stop=True)
            for i in range(2, 8):
                proj_mms(i, _K_ORDER[i])

            # ---------- x LN (half-pipelined chain) ----------
            mx2 = wtile()
            xd = wtile()
            varx = wtile()
            sdx = wtile()
            rsdx = keep.tile([C2, N], F32, tag="rsdx")
            xn = keep.tile([C2, N], BF16, tag="xn")
            if not simple:
                xtmp = wtile()
            for hs in halves:
                nc.scalar.square(mx2[:C2, hs], mx_ps[:C2, hs])
                nc.vector.tensor_tensor(xd[:C2, hs], xsb[:, hs],
                                        mx_ps[:C2, hs], op=SUB)
                nc.vector.tensor_tensor(varx[:C2, hs], mxq_ps[:C2, hs],
                                        mx2[:C2, hs], op=SUB)
                nc.scalar.activation(sdx[:C2, hs], varx[:C2, hs], SQRT,
                                     bias=eps_t[:C2, :])
                nc.vector.reciprocal_approx_fast(rsdx[:, hs], sdx[:C2, hs])
                if simple:
                    nc.vector.tensor_tensor(xn[:, hs], xd[:C2, hs],
                                            rsdx[:, hs], op=MULT)
                else:
                    nc.vector.tensor_tensor(xtmp[:C2, hs], xd[:C2, hs],
                                            rsdx[:, hs], op=MULT)
                    nc.vector.tensor_scalar(xn[:, hs], xtmp[:C2, hs],
                                            g2_sb[:], b2_sb[:],
                                            op0=MULT, op1=ADD)

            # ---------- xn scratch writes + stride-0 broadcasts ----------
            xbe = {}
            _bc_written = set()

            def emit_bcast(which):
                # high priority: the scheduler must not push these behind
                # output DMAs on the same queue -- muls stall on them
                with tc.high_priority():
                    for e0, e1, en in _BCAST:
                        if en != which:
                            continue
                        if which not in _bc_written:
                            eng[which].dma_start(xs_d[which][:], xn[:])
                            _bc_written.add(which)
                        t = xbep.tile([128, (e1 - e0) * N], BF16,
                                      tag=f"xbe{e0}")
                        eng[which].dma_start(
                            t[:],
                            xs_d[which][e0:e1, :].partition_broadcast(128))
                        xbe[(e0, e1)] = t

            emit_bcast("s")
            emit_bcast("g")

            # ---------- q stats, pipelined by 512-column halves ----------
            pb, sq, diff, qnb, dvk = [], [], [], [], []
            for md in range(MD):
                pbt = bp16.tile([128, N], BF16, tag=f"pb{md}")
                pb.append(pbt)
                sqt = bp16.tile([128, N], BF16, tag=f"sq{md}")
                sq.append(sqt)
            mean_ps = ps.tile([128, N], F32, tag="ps")
            msq_ps = ps.tile([128, N], F32, tag="ps")
            mb2 = wtile()
            var = wtile()
            sd = wtile()
            for md in range(MD):
                dft = wtile()
                diff.append(dft)
            rsd = keep.tile([128, N], F32, tag="rsd")
            for md in range(MD):
                qnt = keep.tile([128, N], BF16, tag=f"qn{md}")
                qnb.append(qnt)
            qnb_pool = keep.tile([128, N], BF16, tag="qnp")
            if not simple:
                for md in range(MD):
                    dvt = keep.tile([128, N], F32, tag=f"dv{md}")
                    dvk.append(dvt)

            # ---------- product helpers ----------
            def xbe_of(e0, e1):
                for (b0, b1), t in xbe.items():
                    if b0 <= e0 and e1 <= b1:
                        return t, b0
                raise AssertionError((e0, e1))

            out_view = []
            for md in range(MD):
                ov = out_d[4096 * md : 4096 * (md + 1), :].rearrange(
                    "(p e) n -> p e n", e=32
                )
                out_view.append(ov)

            otile = {}
            for md, j in _ALLOC_ORDER:
                ot = op.tile([128, 4 * N], BF16, tag="ot")
                otile[(md, j)] = ot

            def emit_mul(e_, qsrc, md, e0, e1, h=None):
                j = e0 // 4
                assert e1 <= 4 * (j + 1)
                o = otile[(md, j)]
                xt, b0 = xbe_of(e0, e1)
                e_.tensor_tensor(
                    sub_ap(o, e0 - 4 * j, e1 - 4 * j, h)
                    if h is not None
                    else o[:, (e0 - 4 * j) * N : (e1 - 4 * j) * N],
                    rep_ap(qsrc, e1 - e0, h),
                    sub_ap(xt, e0 - b0, e1 - b0, h),
                    op=MULT)

            def emit_out(md, j):
                o = otile[(md, j)]
                if j == 7:
                    # tail tile: 2-e halves on different engines so the
                    # final transfer after the last mul is half as long
                    e1, e2 = ("s", "a") if md == 0 else ("a", "s")
                    eng[e1].dma_start(out_view[md][:, 28:30, :],
                                      o[:, : 2 * N])
                    eng[e2].dma_start(out_view[md][:, 30:32, :],
                                      o[:, 2 * N :])
                else:
                    eng[_OUT_ENG[md][j]].dma_start(
                        out_view[md][:, 4 * j : 4 * (j + 1), :], o[:])

            # DVE mul order within a half: by e (broadcast arrival order)
            vseq = sorted(
                [(0, c) for c in _MUL_V[0]] + [(1, c) for c in _MUL_V[1]],
                key=lambda mc: (mc[1][0], mc[0]))

            # ---------- stats chain + muls, pipelined by halves ----------
            def emit_pb_sq(hs):
                for md in range(MD):
                    if simple:
                        nc.scalar.copy(pb[md][:, hs], proj[md][:, hs])
                    else:
                        nc.vector.tensor_scalar(pb[md][:, hs],
                                                proj[md][:, hs],
                                                bp_sb[:, md : md + 1], None,
                                                op0=ADD)
                    e_ = nc.vector if md == 0 else nc.gpsimd
                    e_.tensor_tensor(sq[md][:, hs], pb[md][:, hs],
                                     pb[md][:, hs], op=MULT)

            def emit_mms(hs):
                for md in range(MD):
                    nc.tensor.matmul(mean_ps[:, hs], onesq[:], pb[md][:, hs],
                                     start=(md == 0), stop=(md == MD - 1))
                for md in range(MD):
                    nc.tensor.matmul(msq_ps[:, hs], onesq[:], sq[md][:, hs],
                                     start=(md == 0), stop=(md == MD - 1))

            def emit_chain(hs):
                nc.scalar.square(mb2[:, hs], mean_ps[:, hs])
                nc.vector.tensor_tensor(diff[0][:, hs], pb[0][:, hs],
                                        mean_ps[:, hs], op=SUB)
                nc.vector.tensor_tensor(var[:, hs], msq_ps[:, hs],
                                        mb2[:, hs], op=SUB)
                nc.scalar.activation(sd[:, hs], var[:, hs], SQRT,
                                     bias=eps_t[:])
                nc.vector.tensor_tensor(diff[1][:, hs], pb[1][:, hs],
                                        mean_ps[:, hs], op=SUB)
                nc.vector.reciprocal_approx_fast(rsd[:, hs], sd[:, hs])
                if simple:
                    nc.vector.tensor_tensor(qnb[0][:, hs], diff[0][:, hs],
                                            rsd[:, hs], op=MULT)
                    # qn[1] lives only in Pool's copy; DVE md1 muls read it
                    nc.gpsimd.tensor_tensor(qnb_pool[:, hs], diff[1][:, hs],
                                            rsd[:, hs], op=MULT)
                else:
                    for md in range(MD):
                        nc.vector.tensor_tensor(dvk[md][:, hs],
                                                diff[md][:, hs],
                                                rsd[:, hs], op=MULT)
                        nc.vector.tensor_scalar(qnb[md][:, hs],
                                                dvk[md][:, hs],
                                                g1_sb[:, md : md + 1],
                                                b1_sb[:, md : md + 1],
                                                op0=MULT, op1=ADD)
                    nc.gpsimd.tensor_copy(qnb_pool[:, hs], qnb[1][:, hs])

            def emit_muls(hi):
                q1src = qnb_pool if simple else qnb[1]
                with tc.high_priority():
                    for e0, e1 in _MUL_G[1]:
                        emit_mul(nc.gpsimd, qnb_pool, 1, e0, e1, hi)
                    for md, (e0, e1) in vseq:
                        emit_mul(nc.vector,
                                 qnb[0] if md == 0 else q1src, md, e0, e1, hi)

            h0, h1 = halves
            emit_pb_sq(h0)
            emit_mms(h0)
            emit_chain(h0)
            # the whole h1 stats chain is emitted before the Act broadcasts
            # so the bcast DMAs can't block sd-h1 / qnb-h1 on the Act queue,
            # and Pool's full-width muls (needing both qnb_pool halves)
            # unblock as early as possible
            emit_pb_sq(h1)
            emit_mms(h1)
            emit_chain(h1)
            emit_bcast("a")
            emit_muls(0)
            emit_muls(1)

            # output DMAs
            for md, j in _ALLOC_ORDER:
                emit_out(md, j)

    nc.compile()
    return nc


def _host_inputs(q, x, Wp, bp, g1, b1, g2, b2):
    """Build the 8 per-core input maps."""
    import os

    import ml_dtypes

    simple = os.environ.get("HM_SIMPLE", "0") == "1"
    qf = np.asarray(q, dtype=np.float32).reshape(B, C1, N)
    qb = np.ascontiguousarray(qf).astype(ml_dtypes.bfloat16)
    xf = np.ascontiguousarray(
        np.asarray(x, dtype=np.float32).reshape(B, C2, N)
    ).astype(ml_dtypes.bfloat16)
    wpt = np.ascontiguousarray(np.asarray(Wp, dtype=np.float32).T).astype(
        ml_dtypes.bfloat16
    )
    in_maps = []
    for b in range(B):
        m = {
            "q": np.ascontiguousarray(qb[b]),
            "w": wpt,
            "x": np.ascontiguousarray(xf[b]),
        }
        if not simple:
            m["bpc"] = np.ascontiguousarray(
                np.asarray(bp, dtype=np.float32).reshape(MD, 128).T)
            m["g1c"] = np.ascontiguousarray(
                np.asarray(g1, dtype=np.float32).reshape(MD, 128).T)
            m["b1c"] = np.ascontiguousarray(
                np.asarray(b1, dtype=np.float32).reshape(MD, 128).T)
            m["g2r"] = np.ascontiguousarray(
                np.asarray(g2, dtype=np.float32)[:, None])
            m["b2r"] = np.ascontiguousarray(
                np.asarray(b2, dtype=np.float32)[:, None])
        in_maps.append(m)
    return in_maps


def _run(in_maps, trace=False):
    import os

    from concourse.bass_utils import run_bass_kernel_spmd

    key = "nc" + os.environ.get("HM_SIMPLE", "0")
    if key not in _CACHE:
        _CACHE[key] = _build_nc(os.environ.get("HM_SIMPLE", "0") == "1")
    nc = _CACHE[key]
    res = run_bass_kernel_spmd(nc, in_maps, core_ids=list(range(B)), trace=trace)
    return res


def kernel(q, x, Wp, bp, g1, b1, g2, b2):
    import os

    simple = (
        np.allclose(np.asarray(bp), 0)
        and np.allclose(np.asarray(g1), 1)
        and np.allclose(np.asarray(b1), 0)
        and np.allclose(np.asarray(g2), 1)
        and np.allclose(np.asarray(b2), 0)
    )
    os.environ["HM_SIMPLE"] = "1" if simple else "0"
    in_maps = _host_inputs(q, x, Wp, bp, g1, b1, g2, b2)
    res = _run(in_maps, trace=False)
    out = np.stack(
        [
            np.asarray(res.results[b]["out"]).astype(np.float32).reshape(CD, H, W)
            for b in range(B)
        ]
    )
    _CACHE["last_res"] = res
    return out
